# revision 21
# baseline (speedup 1.0000x reference)
"""Trainium2 Bass kernel for the ARCS segment-reduce loss (final).

Math (see reference): per-class weighted segment sums over 2*262144 pixels
-> [19,256] centroids; then z = feat @ cent.T, softmax-entropy per pixel,
confidence-weighted mean -> scalar loss. Output = centroids ++ [loss].

Sharding: data-parallel over pixels, 32768 px/domain/core on 8 cores, with a
[128,38] fp32 AllGather (+ local sum) between the two passes.

Per-core pixel remap: block g, lane p <-> pixel p*B + g (B = 256 blocks).
A pure permutation (all reductions are permutation-invariant) chosen so every
DMA runs long-contiguous and the per-pixel weight / argmax columns are
natural row-major loads (no transposes for them).

Pass 1, per 128-px block (feat streamed as bf16, cast during SWDGE DMA):
  LDW(feat chunk c as stationary [128px, 128d])
    MM rhs=onehotw[128,19]   -> accT_c[128d, 19] (transposed segment sums)
    MM rhs=identity[128,128] -> featT block in PSUM (pass-2 transpose for
                                 free off the same loaded weights)
onehotw = (iota == argmax_col) * w_col in ONE fused tensor_scalar
(DVE/GpSimd split); w values pre-rounded to bf16 because the packed-mode
tensor_scalar truncates its fp32 scalar to 16 bits on HW. Denominators come
from the host (exact f64 bincounts, already needed for the seen-class check).
featT: one 2-bank PSUM tile per 4 blocks, one ACT copy -> fp8 SBUF caches
(both domains resident, 2 x 64KB/partition; pass 2 does zero feature DMA).

Pass 2: z[128px,19] = featT.T @ centT from the fp8 caches, accumulated over
2 d-chunks into [128,24,20] PSUM supertiles; entropy via free-dim reductions
(no max-subtraction needed: |z| <= ~2); Ln/reciprocal deferred to one tail
pass so the ACT Exp table loads once.

Host finishes: centroids = allgathered-sums / denom, loss = -total/n.
Measured: ~303 us on 8 cores, rel err (absmax) ~3.1e-5 vs fp32 reference.
"""

import numpy as np

NUM_CLASS = 19
D_FEAT = 256
N_PIX = 262144
N_CORES = 8
PIX_PER_CORE = N_PIX // N_CORES  # 32768
CB = 16  # blocks per feat DMA chunk / entropy supertile

_BUILD_CACHE = {}


def _build(npix, n_cores):
    import ml_dtypes
    import concourse.bass as bass  # noqa: F401
    import concourse.tile as tile
    from concourse import bacc, mybir

    f32 = mybir.dt.float32
    bf16 = mybir.dt.bfloat16
    fp8 = mybir.dt.float8e4
    EQ = mybir.AluOpType.is_equal
    MUL = mybir.AluOpType.mult
    ADD = mybir.AluOpType.add
    SUB = mybir.AluOpType.subtract
    Exp = mybir.ActivationFunctionType.Exp
    Ln = mybir.ActivationFunctionType.Ln
    X = mybir.AxisListType.X

    C = NUM_CLASS
    B = npix // 128            # blocks per domain (full: 256)
    assert npix % (128 * CB) == 0
    NCH = B // CB              # chunks per domain
    B4 = B // 4                # featT groups of 4 blocks

    nc = bacc.Bacc("TRN2", target_bir_lowering=False, debug=False,
                   num_devices=n_cores)

    sfeat = nc.dram_tensor("sfeat", [npix, D_FEAT], f32, kind="ExternalInput")
    tfeat = nc.dram_tensor("tfeat", [npix, D_FEAT], f32, kind="ExternalInput")
    tconf = nc.dram_tensor("tconf", [npix], f32, kind="ExternalInput")
    sam = nc.dram_tensor("sam", [npix], mybir.dt.int32, kind="ExternalInput")
    tam = nc.dram_tensor("tam", [npix], mybir.dt.int32, kind="ExternalInput")
    smask = nc.dram_tensor("smask", [npix], mybir.dt.uint8, kind="ExternalInput")
    denomv = nc.dram_tensor("denomv", [C, 1], f32, kind="ExternalInput")

    sred_out = nc.dram_tensor("sred", [128, 2 * C], f32,
                              kind="ExternalOutput")
    accw_out = nc.dram_tensor("accw", [128, 1], f32, kind="ExternalOutput")

    ident_bf_d = nc.inline_tensor(np.eye(128).astype(ml_dtypes.bfloat16),
                                  "ident_bf")
    ident_f32_d = nc.inline_tensor(np.eye(128, dtype=np.float32), "ident_f32")
    iota_np = np.concatenate([np.arange(C), [100.0]]).astype(ml_dtypes.bfloat16)
    iota_d = nc.inline_tensor(np.tile(iota_np, (128, 1)), "iota_c")

    with tile.TileContext(nc) as tc:
        with (
            tc.tile_pool(name="const", bufs=1) as const_pool,
            tc.tile_pool(name="persist", bufs=1) as persist,
            tc.tile_pool(name="cache", bufs=1) as cache_pool,
            tc.tile_pool(name="feat", bufs=5) as feat_pool,
            tc.tile_pool(name="oh", bufs=8) as oh_pool,
            tc.tile_pool(name="ent", bufs=2) as ent_pool,
            tc.tile_pool(name="small", bufs=1) as small_pool,
            tc.tile_pool(name="psacc", bufs=1, space="PSUM") as psacc_pool,
            tc.tile_pool(name="pstr", bufs=3, space="PSUM") as pstr_pool,
            tc.tile_pool(name="dram", bufs=1, space="DRAM") as dram_pool,
        ):
            ident_bf = const_pool.tile([128, 128], bf16)
            nc.gpsimd.dma_start(ident_bf[:], ident_bf_d[:])
            ident_f32 = const_pool.tile([128, 128], f32)
            nc.gpsimd.dma_start(ident_f32[:], ident_f32_d[:])
            iota = const_pool.tile([128, C + 1], bf16)
            nc.gpsimd.dma_start(iota[:], iota_d[:])
            ones_bf = const_pool.tile([128, 1], bf16)
            nc.vector.memset(ones_bf[:], 1.0)
            ones_f32r = const_pool.tile([1, 128], f32)
            nc.vector.memset(ones_f32r[:], 1.0)

            # per-pixel weights / argmax: natural row-major loads, wT[p, g]
            # = w[p*B + g]
            wT_s = persist.tile([128, B], f32)
            raw_m = small_pool.tile([128, B], mybir.dt.uint8, name="raw_m")
            nc.gpsimd.dma_start(raw_m[:], smask[:].rearrange("(p g) -> p g", g=B))
            nc.vector.tensor_copy(wT_s[:], raw_m[:])
            wT_t = persist.tile([128, B], f32)
            raw_c = small_pool.tile([128, B], f32, name="raw_c")
            nc.gpsimd.dma_start(raw_c[:], tconf[:].rearrange("(p g) -> p g", g=B))
            # round w_t to bf16-exact values: the packed-mode tensor_scalar
            # truncates its fp32 scalar operand to 16 bits on hardware
            wt_bf = small_pool.tile([128, B], bf16, name="wt_bf")
            nc.vector.tensor_scalar(wt_bf[:], raw_c[:], -1.0, 1.0, MUL, ADD)
            nc.vector.tensor_copy(wT_t[:], wt_bf[:])
            amT_s = persist.tile([128, B], f32)
            raw_s = small_pool.tile([128, B], mybir.dt.int32, name="raw_s")
            nc.gpsimd.dma_start(raw_s[:], sam[:].rearrange("(p g) -> p g", g=B))
            nc.vector.tensor_copy(amT_s[:], raw_s[:])
            amT_t = persist.tile([128, B], f32)
            raw_t = small_pool.tile([128, B], mybir.dt.int32, name="raw_t")
            nc.gpsimd.dma_start(raw_t[:], tam[:].rearrange("(p g) -> p g", g=B))
            nc.vector.tensor_copy(amT_t[:], raw_t[:])

            # persistent accumulators
            accT0 = psacc_pool.tile([128, C], f32)
            accT1 = psacc_pool.tile([128, C], f32)
            src_cache = cache_pool.tile([128, B4, 1024], fp8)
            tgt_cache = cache_pool.tile([128, B4, 1024], fp8)
            S_all = persist.tile([128, 2 * B], f32)
            D_all = persist.tile([128, 2 * B], f32)

            # ---------------- pass 1 ----------------
            warm_ps = pstr_pool.tile([128, 8, 128], f32, name="warm_ps",
                                     tag="bank")
            first = True
            for dom, (feat, amT, wT) in enumerate(
                    ((sfeat, amT_s, wT_s), (tfeat, amT_t, wT_t))):
                fv = feat[:].rearrange("(p c b) d -> c p b d", c=NCH, b=CB)
                for ch in range(NCH):
                    ft = feat_pool.tile([128, CB, D_FEAT], bf16, name="ft1",
                                        tag="ft1")
                    nc.gpsimd.dma_start(ft[:], fv[ch])
                    if dom == 0 and ch == 0:
                        # ~6us dense matmul burst to flip the PE HAM clock
                        # gate to 8/8 before the real (small-N) matmuls
                        for wi in range(32):
                            nc.tensor.matmul(
                                warm_ps[:, 0:2, :], ft[:, wi % CB, 0:128],
                                ft[:, (wi + 1) % CB, :],
                                start=True, stop=True)
                    for jq in range(CB // 4):  # featT groups of 4 blocks
                        bankA = pstr_pool.tile([128, 8, 128], f32, name="bankA",
                                               tag="bank")
                        for j4 in range(4):
                            j = jq * 4 + j4
                            g = ch * CB + j
                            last = (dom == 1 and ch == NCH - 1 and j == CB - 1)
                            oh = oh_pool.tile([128, C + 1], bf16, name="oh",
                                              tag="oh")
                            oh_eng = nc.gpsimd if g % 5 < 2 else nc.vector
                            oh_eng.tensor_scalar(oh[:], iota[:],
                                                 amT[:, g:g + 1],
                                                 wT[:, g:g + 1], EQ, MUL)
                            for c in range(2):
                                fslice = ft[:, j, c * 128:(c + 1) * 128]
                                accT = accT0 if c == 0 else accT1
                                nc.tensor.matmul(accT[:], fslice,
                                                 oh[:, 0:C],
                                                 start=first, stop=last)
                                nc.tensor.matmul(bankA[:, j4 * 2 + c, :],
                                                 fslice, ident_bf[:],
                                                 start=True, stop=True)
                            first = False
                        # evacuate featT: 4 blocks -> one [128,1024] fp8 tile
                        g0 = ch * CB + jq * 4
                        cache = tgt_cache if dom == 1 else src_cache
                        nc.scalar.copy(cache[:, g0 // 4, :], bankA[:])

            # ---------------- AllReduce [128, 39] ----------------
            cc_sb = persist.tile([128, 2 * C], f32)
            nc.vector.tensor_copy(cc_sb[:, 0:C], accT0[:])
            nc.vector.tensor_copy(cc_sb[:, C:2 * C], accT1[:])
            cc_in = dram_pool.tile([128, 2 * C], f32)
            cc_addr = "Shared" if n_cores > 4 else "Local"
            cc_out = dram_pool.tile([n_cores * 128, 2 * C], f32,
                                    addr_space=cc_addr)
            nc.gpsimd.dma_start(cc_in[:], cc_sb[:])
            nc.gpsimd.collective_compute(
                "AllGather", mybir.AluOpType.bypass,
                replica_groups=[list(range(n_cores))],
                ins=[cc_in.opt()], outs=[cc_out.opt()])
            gat = persist.tile([128, n_cores, 2 * C], f32)
            nc.gpsimd.dma_start(
                gat[:], cc_out[:].rearrange("(k p) c -> p k c", p=128))
            allred = persist.tile([128, 2 * C], f32)
            nc.vector.tensor_tensor(allred[:], gat[:, 0, :], gat[:, 1, :], ADD)
            for k in range(2, n_cores):
                nc.vector.tensor_tensor(allred[:], allred[:], gat[:, k, :], ADD)
            nc.sync.dma_start(sred_out[:], allred[:])

            # centT[d, c] = accT[d, c] / denom[c]  (bf16, for the z matmuls)
            den_sb = small_pool.tile([C, 1], f32, name="den_sb")
            nc.gpsimd.dma_start(den_sb[:], denomv[:])
            rec_col = small_pool.tile([C, 1], f32, name="rec_col")
            nc.vector.reciprocal(rec_col[:], den_sb[:])
            rec_ps = pstr_pool.tile([1, C], f32, name="rec_ps", tag="bank")
            nc.tensor.transpose(rec_ps[:], rec_col[:], ident_f32[0:C, 0:C])
            rec_row = small_pool.tile([1, C], f32, name="rec_row")
            nc.vector.tensor_copy(rec_row[:], rec_ps[:])
            recb_ps = pstr_pool.tile([128, C], f32, name="recb_ps", tag="bank")
            nc.tensor.matmul(recb_ps[:], ones_f32r[:], rec_row[:],
                             start=True, stop=True)
            rec_tile = small_pool.tile([128, C], f32, name="rec_tile")
            nc.vector.tensor_copy(rec_tile[:], recb_ps[:])
            centT = persist.tile([128, 2, C], bf16)
            nc.vector.tensor_tensor(centT[:, 0, :], allred[:, 0:C], rec_tile[:],
                                    MUL)
            nc.vector.tensor_tensor(centT[:, 1, :], allred[:, C:2 * C],
                                    rec_tile[:], MUL)

            # ---------------- pass 2 ----------------
            # supertile of up to 24 blocks = one PSUM bank of [128, ST, 20]
            groups = []
            g0 = 0
            while g0 < B:
                st = min(24, B - g0)
                groups.append((g0, st))
                g0 += st
            for dom in (0, 1):
                cache = tgt_cache if dom == 1 else src_cache
                for g0, st in groups:
                    zps = pstr_pool.tile([128, 24, 20], f32, name="zps",
                                         tag="bank")
                    for j in range(st):
                        g = g0 + j
                        for c in range(2):
                            s = ((g % 4) * 2 + c) * 128
                            lhsT = cache[:, g // 4, s:s + 128]
                            nc.tensor.matmul(zps[:, j, 0:C], lhsT,
                                             centT[:, c, :],
                                             start=(c == 0), stop=(c == 1))
                    zv = zps[:, 0:st, 0:C]
                    e = ent_pool.tile([128, 24 * C], f32, name="e", tag="e")
                    nc.scalar.activation(e[:, 0:st * C], zv, Exp)
                    ezz = ent_pool.tile([128, 24 * C], f32, name="ezz",
                                        tag="ezz")
                    nc.vector.tensor_tensor(ezz[:, 0:st * C], e[:, 0:st * C],
                                            zv, MUL)
                    col = dom * B + g0
                    nc.vector.reduce_sum(
                        S_all[:, col:col + st],
                        e[:, 0:st * C].rearrange("p (a b) -> p a b", b=C),
                        axis=X)
                    nc.vector.reduce_sum(
                        D_all[:, col:col + st],
                        ezz[:, 0:st * C].rearrange("p (a b) -> p a b", b=C),
                        axis=X)

            # ---------------- tail: ent = (D/S - ln S) * w ----------------
            logS = persist.tile([128, 2 * B], f32)
            nc.scalar.activation(logS[:], S_all[:], Ln)
            rS = persist.tile([128, 2 * B], f32)
            nc.vector.reciprocal(rS[:], S_all[:])
            ent_all = persist.tile([128, 2 * B], f32)
            nc.vector.tensor_tensor(ent_all[:], D_all[:], rS[:], MUL)
            nc.vector.tensor_tensor(ent_all[:], ent_all[:], logS[:], SUB)
            nc.vector.tensor_tensor(ent_all[:, 0:B], ent_all[:, 0:B], wT_s[:],
                                    MUL)
            nc.vector.tensor_tensor(ent_all[:, B:2 * B], ent_all[:, B:2 * B],
                                    wT_t[:], MUL)
            acc = persist.tile([128, 1], f32)
            nc.vector.reduce_sum(acc[:], ent_all[:], axis=X)
            nc.sync.dma_start(accw_out[:], acc[:])

    nc.compile()
    return nc


def get_nc(npix=PIX_PER_CORE, n_cores=N_CORES):
    key = (npix, n_cores)
    if key not in _BUILD_CACHE:
        _BUILD_CACHE[key] = _build(npix, n_cores)
    return _BUILD_CACHE[key]


def make_in_maps(source_feat, target_feat, target_conf, source_argmax,
                 target_argmax, source_mask, denom, n_cores=N_CORES):
    npix = source_feat.shape[0] // n_cores
    maps = []
    for k in range(n_cores):
        s = slice(k * npix, (k + 1) * npix)
        maps.append({
            "denomv": np.asarray(denom, np.float32).reshape(NUM_CLASS, 1),
            "sfeat": np.ascontiguousarray(source_feat[s]),
            "tfeat": np.ascontiguousarray(target_feat[s]),
            "tconf": np.ascontiguousarray(target_conf[s]),
            "sam": np.ascontiguousarray(source_argmax[s]),
            "tam": np.ascontiguousarray(target_argmax[s]),
            "smask": np.ascontiguousarray(source_mask[s]).view(np.uint8),
        })
    return maps


def finish_on_host(sred, acc_total, source_mask, denom):
    """sred: [128, 38] allreduced (accT0 | accT1); denom: host bincounts."""
    C = NUM_CLASS
    sum_c = np.concatenate([sred[:, 0:C], sred[:, C:2 * C]], axis=0).T
    denom = np.asarray(denom, np.float32).reshape(C)
    seen = denom > 0
    cent = np.where(seen[:, None],
                    sum_c / np.maximum(denom, 1e-12)[:, None],
                    np.float32(np.inf)).astype(np.float32)
    n = np.float32(float(source_mask.sum()) + source_mask.shape[0])
    loss = np.float32(-(acc_total / n))
    return np.concatenate([cent.reshape(-1), np.asarray([loss], np.float32)])


def _numpy_reference(source_feat, target_feat, target_conf, source_argmax,
                     target_argmax, source_mask):
    """Exact numpy replica of the reference (fallback path)."""
    C = NUM_CLASS
    w_s = source_mask.astype(np.float32)
    w_t = 1.0 - target_conf
    sum_c = np.zeros((C, D_FEAT), np.float32)
    np.add.at(sum_c, source_argmax, source_feat * w_s[:, None])
    np.add.at(sum_c, target_argmax, target_feat * w_t[:, None])
    denom = (np.bincount(source_argmax, weights=w_s, minlength=C)
             + np.bincount(target_argmax, weights=w_t, minlength=C)).astype(
                 np.float32)
    seen = denom > 0
    cent = np.where(seen[:, None], sum_c / np.maximum(denom, 1e-12)[:, None],
                    np.inf).astype(np.float32)
    cent_safe = np.where(seen[:, None], cent, 0.0).astype(np.float32)

    def ent(feat):
        z = feat @ cent_safe.T
        z = np.where(seen[None, :], z, -np.inf)
        zmax = z.max(axis=1, keepdims=True)
        e = np.exp(z - zmax)
        s = e.sum(axis=1, keepdims=True)
        logp = z - (zmax + np.log(s))
        p = e / s
        return np.sum(np.where(seen[None, :], p * logp, 0.0), axis=1)

    total = float((w_s * ent(source_feat)).sum()
                  + (w_t * ent(target_feat)).sum())
    n = float(w_s.sum()) + source_feat.shape[0]
    loss = np.float32(-total / n)
    return np.concatenate([cent.reshape(-1), np.asarray([loss], np.float32)])


def kernel(source_feat, target_feat, target_conf, source_argmax, target_argmax,
           source_mask, _trace=False):
    source_feat = np.asarray(source_feat, np.float32)
    target_feat = np.asarray(target_feat, np.float32)
    target_conf = np.asarray(target_conf, np.float32)
    source_argmax = np.asarray(source_argmax, np.int32)
    target_argmax = np.asarray(target_argmax, np.int32)
    source_mask = np.asarray(source_mask)

    d_host = (np.bincount(source_argmax,
                          weights=source_mask.astype(np.float64),
                          minlength=NUM_CLASS)
              + np.bincount(target_argmax,
                            weights=(1.0 - target_conf).astype(np.float64),
                            minlength=NUM_CLASS))
    if not np.all(d_host > 0):
        return _numpy_reference(source_feat, target_feat, target_conf,
                                source_argmax, target_argmax, source_mask)

    from concourse.bass_utils import run_bass_kernel_spmd

    nc = get_nc()
    in_maps = make_in_maps(source_feat, target_feat, target_conf,
                           source_argmax, target_argmax, source_mask, d_host)
    res = run_bass_kernel_spmd(nc, in_maps, list(range(N_CORES)),
                               trace=_trace)
    sred = res.results[0]["sred"]
    acc_total = float(sum(r["accw"].astype(np.float64).sum()
                          for r in res.results))
    out = finish_on_host(sred, acc_total, source_mask, d_host)
    if _trace:
        return out, res
    return out
